# revision 50
# baseline (speedup 1.0000x reference)
"""Trainium2 Bass kernel for nn_DecoderRNN (LSTM decoder + pointer-generator).

Strategy (8 NeuronCores, SPMD, no collectives):
  - LSTM + attention replicated on every core over the full batch (the LSTM
    recurrence cost is weight-bound, independent of batch, so sharding batch
    would not reduce it; replication avoids collectives entirely).
  - The 30000-wide output matmul is vocab-sharded: core c computes output
    columns [c*3750, (c+1)*3750) and the host concatenates.
  - Pointer path is rewritten: outputs_pointer = (aw @ onehot(enc_in)) @ converter
    so the 30000-wide gather of `converter[encoder_input]` never materializes.
  - p_gen combine is folded into the matmul inputs:
        out = (H*p) @ W.T + p x linear_b + ((1-p)*S) @ converter
  - Softmax normalization is applied to exp(scores) directly (no max-subtract;
    scores are O(25) so fp32 exp is safe).

Layouts are "transposed": rows of the output (time-major index c = t*B + b)
live on the free axis; hidden/gate/vocab dims live on partitions.
"""

import os

import numpy as np

B = 32
L = 64
EMB = 256
HID = 512
VOCAB = 30000
NOBJ = 91
NCORES = 8
VL = VOCAB // NCORES  # 3750

F32 = "float32"
BF16 = "bfloat16"

_CACHE = {}
LAST_RESULT = None


# ---------------------------------------------------------------------------
# walrus CTRL-encoding legalization: hoist extra sem-waits onto same-engine NOPs
def _split_multi_waits(nc, mybir, max_waits=1):
    n_fix = 0
    for f in nc.m.functions:
        for block in f.blocks:
            insts = list(block.instructions)
            out = []
            changed = False
            for inst in insts:
                si = inst.sync_info
                waits = list(si.on_wait) if si is not None else []
                if len(waits) > max_waits:
                    extra = waits[:-max_waits]
                    keep = waits[-max_waits:]
                    chunks = [
                        extra[i : i + max_waits]
                        for i in range(0, len(extra), max_waits)
                    ]
                    for ci, chunk in enumerate(chunks):
                        nop = mybir.InstNoOp(
                            name=f"{inst.name}-waitfix-{ci}",
                            engine=inst.engine,
                            sync_info=mybir.SyncInfo(on_wait=chunk, on_update=[]),
                            bass_nofuse=True,
                        )
                        nc.register_instruction(nop)
                        out.append(nop)
                    inst.sync_info = mybir.SyncInfo(
                        on_wait=keep, on_update=list(si.on_update)
                    )
                    n_fix += 1
                    changed = True
                out.append(inst)
            if changed:
                block.instructions = out
    return n_fix


# ---------------------------------------------------------------------------
def _build(T):
    """Build the SPMD Bass program for sequence length T. Returns (nc, meta)."""
    import concourse.bass as bass
    import concourse.tile as tile
    from concourse import mybir

    dt = mybir.dt
    AF = mybir.ActivationFunctionType
    ALU = mybir.AluOpType

    R = T * B  # number of output rows
    RP = ((R + 127) // 128) * 128  # padded to full partition tiles
    MT = RP // 128  # number of 128-row output tiles
    NCH = (VL + 511) // 512  # vocab N-chunks per core

    nc = bass.Bass()

    # ---------------- DRAM I/O ----------------
    d_xt = nc.dram_tensor("xt", [2, 128, R], dt.bfloat16, kind="ExternalInput")
    d_wih = nc.dram_tensor("wih", [2, 128, 4 * HID], dt.bfloat16, kind="ExternalInput")
    d_bias = nc.dram_tensor("biaspm", [128, 16], dt.float32, kind="ExternalInput")
    d_whh = nc.dram_tensor("whh", [4, 128, 4 * HID], dt.bfloat16, kind="ExternalInput")
    d_encb = nc.dram_tensor("encb", [L, B * EMB], dt.bfloat16, kind="ExternalInput")
    d_enct = nc.dram_tensor("enct", [2, 128, B * L], dt.float32, kind="ExternalInput")
    d_oh = nc.dram_tensor("oh", [L, B * NOBJ], dt.float32, kind="ExternalInput")
    d_mask = nc.dram_tensor("mask01", [L, B], dt.float32, kind="ExternalInput")
    d_eye = nc.dram_tensor("eye", [128, 128], dt.bfloat16, kind="ExternalInput")
    d_awt = nc.dram_tensor("awt", [4, 128, EMB], dt.bfloat16, kind="ExternalInput")
    d_attnb = nc.dram_tensor("attnb", [128, 2], dt.float32, kind="ExternalInput")
    d_pge = nc.dram_tensor("pge", [128, 2], dt.bfloat16, kind="ExternalInput")
    d_pgd = nc.dram_tensor("pgd", [128, 4], dt.bfloat16, kind="ExternalInput")
    d_pb = nc.dram_tensor("pb", [1, 1], dt.float32, kind="ExternalInput")
    d_wt = nc.dram_tensor("wt", [4, 128, VL], dt.bfloat16, kind="ExternalInput")
    d_conv = nc.dram_tensor("conv", [NOBJ + 6, VL], dt.bfloat16, kind="ExternalInput")
    d_out = nc.dram_tensor("out", [R, VL], dt.float32, kind="ExternalOutput")

    with tile.TileContext(nc) as tc:
        with (
            tc.tile_pool(name="pers", bufs=1) as pers,
            tc.tile_pool(name="arena", bufs=1) as arena,
            tc.tile_pool(name="small", bufs=3) as small,
            tc.tile_pool(name="omp", bufs=4) as omp,
            tc.tile_pool(name="psA", bufs=3, space="PSUM") as psA,
            tc.tile_pool(name="psB", bufs=2, space="PSUM") as psB,
        ):
            # ---------- persistent tiles ----------
            HT = pers.tile([128, 4, 32 * (T + 1)], dt.bfloat16, tag="HT")
            Hs = pers.tile([128, 4, RP], dt.bfloat16, tag="Hs")
            Ss2 = pers.tile([NOBJ + 6, RP], dt.bfloat16, tag="Ss")
            Ssu = pers.tile([NOBJ, RP], dt.float32, tag="Ssu")
            ptm = pers.tile([1, RP], dt.bfloat16, tag="ptm")
            sstm = pers.tile([1, RP], dt.float32, tag="sstm")
            Qsb = pers.tile([128, 2, R], dt.float32, tag="Qsb")
            Esb = pers.tile([L, R], dt.float32, tag="Esb")
            Ebf = pers.tile([L, R], dt.bfloat16, tag="Ebf")
            ctxsb = pers.tile([128, 2, R], dt.bfloat16, tag="ctxsb")
            rinv = pers.tile([1, R], dt.float32, tag="rinv")
            pch = pers.tile([1, 5 * R], dt.float32, tag="pch")
            enct_sb = pers.tile([128, 2, B * L], dt.float32, tag="enct")
            oh_sb = pers.tile([L, B * NOBJ], dt.float32, tag="oh")
            mask_sb = pers.tile([L, B], dt.float32, tag="mask")
            eye_sb = pers.tile([128, 128], dt.bfloat16, tag="eye")
            awt_sb = pers.tile([128, 4, EMB], dt.bfloat16, tag="awt")
            attnb_sb = pers.tile([128, 2], dt.float32, tag="attnb")
            pge_sb = pers.tile([128, 2], dt.bfloat16, tag="pge")
            pgd_sb = pers.tile([128, 4], dt.bfloat16, tag="pgd")
            pb_sb = pers.tile([1, 1], dt.float32, tag="pb")
            bias_sb = pers.tile([128, 16], dt.float32, tag="biaspm")
            conv_sb = pers.tile([NOBJ + 6, VL], dt.bfloat16, tag="conv")
            ones_f = pers.tile([1, 128], dt.float32, tag="ones_f")
            ones64 = pers.tile([L, 1], dt.float32, tag="ones64")
            ones_bf = pers.tile([1, 128], dt.bfloat16, tag="ones_bf")

            nc.vector.memset(ones_f[:], 1.0)
            nc.vector.memset(ones64[:], 1.0)
            nc.vector.memset(ones_bf[:], 1.0)

            # ---------- phase-0 DMAs (sync/HWDGE) ----------
            xt_sb = arena.tile([128, 2, R], dt.bfloat16, tag="slotD")
            wih_sb = arena.tile([128, 2, 4 * HID], dt.bfloat16, tag="slotC")
            Gx = arena.tile([128, 16, R], dt.bfloat16, tag="slotA")
            whh_sb = arena.tile([128, 4, 4 * HID], dt.bfloat16, tag="slotB")

            # spread input loads over three DMA queues so the Gx inputs
            # (sync queue) are not stuck behind the big attention loads
            # critical-path loads only (Gx then LSTM); everything needed for
            # the attention/vocab phases is issued after the Gx matmuls so it
            # doesn't steal DMA bandwidth from the startup
            for k in range(2):
                nc.sync.dma_start(xt_sb[:, k], d_xt[k])
                nc.sync.dma_start(wih_sb[:, k], d_wih[k])
            nc.sync.dma_start(bias_sb[:], d_bias[:])
            nc.sync.dma_start(eye_sb[:], d_eye[:])
            nc.scalar.dma_start(whh_sb[:], d_whh[:].rearrange("k p m -> p k m"))

            # ---------- Gx = W_ih' @ X^T  (+bias on copy-out), bf16 ----------
            # Gx layout [128, 16 gate-tiles, R]; column c = t*B + b.
            gx_gate_mm = None
            for m in range(16):
                ps = psB.tile([128, 1024], dt.float32, tag="big")
                for n0 in range(0, R, 512):
                    nn = min(512, R - n0)
                    for k in range(2):
                        _mm = nc.tensor.matmul(
                            ps[:, n0 : n0 + nn],
                            wih_sb[:, k, m * 128 : (m + 1) * 128],
                            xt_sb[:, k, n0 : n0 + nn],
                            start=(k == 0),
                            stop=(k == 1),
                        )
                        if gx_gate_mm is None:
                            gx_gate_mm = _mm
                if m % 2 == 0:
                    nc.scalar.activation(
                        Gx[:, m, 0:R], ps[:, :R],
                        AF.Identity, bias=bias_sb[:, m : m + 1],
                    )
                else:
                    nc.vector.tensor_scalar(
                        out=Gx[:, m, 0:R],
                        in0=ps[:, :R],
                        scalar1=bias_sb[:, m : m + 1],
                        scalar2=None,
                        op0=ALU.add,
                    )

            # attention/vocab-phase loads (needed ~150us later); explicitly
            # held back behind the first Gx matmul so they don't steal DMA
            # bandwidth from the startup-critical loads
            import bass_rust as _br

            late = []
            late.append(nc.scalar.dma_start(enct_sb[:], d_enct[:].rearrange("k p n -> p k n")))
            late.append(nc.gpsimd.dma_start(oh_sb[:], d_oh[:]))
            late.append(nc.gpsimd.dma_start(mask_sb[:], d_mask[:]))
            late.append(nc.gpsimd.dma_start(awt_sb[:], d_awt[:].rearrange("k p m -> p k m")))
            late.append(nc.gpsimd.dma_start(attnb_sb[:], d_attnb[:]))
            late.append(nc.gpsimd.dma_start(pge_sb[:], d_pge[:]))
            late.append(nc.gpsimd.dma_start(pgd_sb[:], d_pgd[:]))
            late.append(nc.gpsimd.dma_start(pb_sb[:], d_pb[:]))
            late.append(nc.gpsimd.dma_start(conv_sb[:], d_conv[:]))
            # enc_bf reuses xt's arena slot; starts after Gx consumes xt.
            encb_sb = arena.tile([L, B * EMB], dt.bfloat16, tag="slotD")
            late.append(nc.gpsimd.dma_start(encb_sb[:], d_encb[:]))
            for dma in late:
                _br.add_dep_helper(
                    dma.ins, gx_gate_mm.ins, True,
                    "hold noncritical DMA behind Gx start",
                )

            # ---------- LSTM over T steps ----------
            # HT column slots: slot 0 = h_{-1} = 0; step t writes slot t+1.
            nc.vector.memset(HT[:, :, 0:32], 0.0)

            # Per step: PE accumulates gates = copy(Gx_t) + W_hh @ h_{t-1}
            # into one PSUM bank; the nonlinear tail is split into two
            # hidden-halves so it pipelines against the next step's matmuls
            # (the k∈{0,1} matmuls only need the first half of h_t).
            cprev = None
            for t in range(T):
                ht_prev = HT[:, :, t * 32 : (t + 1) * 32]
                ps = psA.tile([128, 512], dt.float32, tag="psA")
                # gates := Gx_t  (identity matmul; Gx is ready long before h)
                nc.tensor.matmul(
                    ps[:, :],
                    eye_sb[:],
                    Gx[:, :, t * 32 : (t + 1) * 32],
                    start=True,
                    stop=True,
                )
                # += W_hh @ h_{t-1}; k-major so the k-th pass depends only on
                # the k-th quarter of h, which the previous tail emits first
                for k in range(4):
                    for m in range(16):
                        nc.tensor.matmul(
                            ps[:, m * 32 : (m + 1) * 32],
                            whh_sb[:, k, m * 128 : (m + 1) * 128],
                            ht_prev[:, k, :],
                            start=False,
                            stop=(k == 3),
                            skip_group_check=True,
                        )
                cnew = small.tile([128, 4, 32], dt.float32, tag="c_t")
                for hx in (0, 1):
                    # gate chunk layout: m = 4*gate + tile, gates [i,f,o,g];
                    # half hx covers tiles {2hx, 2hx+1} of each gate.
                    # sigmoid first: B = f*c (DVE) runs while tanh(g) is on ACT
                    sig = small.tile([128, 3, 2, 32], dt.float32, tag=f"sig{hx}")
                    nc.scalar.activation(
                        sig[:],
                        ps[:, :].rearrange("p (g u x) -> p g u x", g=4, u=4)[
                            :, 0:3, 2 * hx : 2 * hx + 2, :
                        ],
                        AF.Sigmoid,
                    )
                    tg = small.tile([128, 2, 32], dt.float32, tag=f"tg{hx}")
                    nc.scalar.activation(
                        tg[:],
                        ps[:, (12 + 2 * hx) * 32 : (14 + 2 * hx) * 32].rearrange(
                            "p (u x) -> p u x", u=2
                        ),
                        AF.Tanh,
                    )
                    if cprev is not None:
                        b_t = small.tile([128, 2, 32], dt.float32, tag=f"b_t{hx}")
                        nc.vector.tensor_mul(
                            b_t[:], sig[:, 1], cprev[:, 2 * hx : 2 * hx + 2]
                        )
                    a_t = small.tile([128, 2, 32], dt.float32, tag=f"a_t{hx}")
                    nc.vector.tensor_mul(a_t[:], sig[:, 0], tg[:])
                    if cprev is None:
                        nc.vector.tensor_copy(cnew[:, 2 * hx : 2 * hx + 2], a_t[:])
                    else:
                        nc.vector.tensor_add(
                            cnew[:, 2 * hx : 2 * hx + 2], a_t[:], b_t[:]
                        )
                    tc_t = small.tile([128, 2, 32], dt.float32, tag=f"tc_t{hx}")
                    nc.scalar.activation(
                        tc_t[:], cnew[:, 2 * hx : 2 * hx + 2], AF.Tanh
                    )
                    # emit h per k-tile so the next step's k-th matmul pass
                    # can start as soon as its slice exists
                    for u in (0, 1):
                        nc.vector.tensor_mul(
                            HT[
                                :,
                                2 * hx + u : 2 * hx + u + 1,
                                (t + 1) * 32 : (t + 2) * 32,
                            ],
                            sig[:, 2, u : u + 1],
                            tc_t[:, u : u + 1],
                        )
                cprev = cnew

            HTv = HT[:, :, 32 : 32 + R]  # h_1..h_T columns, time-major

            # late big DMAs (SWDGE queue) into freed arena slots
            Wt_sb = arena.tile([128, 4, VL], dt.bfloat16, tag="slotA")
            nc.gpsimd.dma_start(Wt_sb[:], d_wt[:].rearrange("k p v -> p k v"))

            # ---------- attention ----------
            # Q^T [256, R] = attn_W @ H^T   (bf16, psum f32)
            for m in range(2):
                qp = psB.tile([128, 1024], dt.float32, tag="big")
                for n0 in range(0, R, 512):
                    nn = min(512, R - n0)
                    for k in range(4):
                        nc.tensor.matmul(
                            qp[:, n0 : n0 + nn],
                            awt_sb[:, k, m * 128 : (m + 1) * 128],
                            HTv[:, k, :][:, n0 : n0 + nn],
                            start=(k == 0),
                            stop=(k == 3),
                        )
                nc.vector.tensor_scalar(
                    out=Qsb[:, m, :],
                    in0=qp[:, :R],
                    scalar1=attnb_sb[:, m : m + 1],
                    scalar2=None,
                    op0=ALU.add,
                )

            # scores^T [64, R] batch-major columns d = b*32 + t
            sc = psB.tile([L, R], dt.float32, tag="big")
            for b in range(B):
                for k in range(2):
                    nc.tensor.matmul(
                        sc[:, b * T : (b + 1) * T],
                        enct_sb[:, k, b * L : (b + 1) * L],
                        Qsb[:, k, :].rearrange("p (t bb) -> p bb t", bb=B)[:, b, :],
                        start=(k == 0),
                        stop=(k == 1),
                    )
            # E = exp(scores)  [64, R] f32  -- UNNORMALIZED; the softmax
            # denominator is folded in later (everything downstream is linear
            # in aw until p_gen, which gets an explicit rinv factor).
            # Masking is multiplicative: exp(s + m01*-inf) == exp(s)*m01.
            nc.scalar.activation(Esb[:], sc[:, :], AF.Exp)
            mb = mask_sb[:, :]
            nc.vector.tensor_mul(
                Esb[:, :].rearrange("p (bb t) -> p bb t", t=T),
                Esb[:, :].rearrange("p (bb t) -> p bb t", t=T),
                bass.AP(tensor=mb.tensor, offset=mb.offset,
                        ap=[list(mb.ap[0]), [1, B], [0, T]]),
            )
            nc.vector.tensor_copy(Ebf[:], Esb[:])

            # column sums -> rinv (the 1/x is slow DVE work; it overlaps the
            # ctx / S matmuls below, which use unnormalized aw)
            cs = psB.tile([1, R], dt.float32, tag="big")
            for n0 in range(0, R, 512):
                nn = min(512, R - n0)
                nc.tensor.matmul(
                    cs[:, n0 : n0 + nn], ones64[:], Esb[:, n0 : n0 + nn],
                )
            nc.vector.reciprocal(rinv[:], cs[:, :])

            # ctx_un^T [256, R] bf16 (batch-major cols), unnormalized aw
            for m in range(2):
                cp = psB.tile([128, 1024], dt.float32, tag="big")
                for b in range(B):
                    nc.tensor.matmul(
                        cp[:, b * T : (b + 1) * T],
                        encb_sb[:, b * EMB + m * 128 : b * EMB + (m + 1) * 128],
                        Ebf[:, b * T : (b + 1) * T],
                    )
                nc.scalar.copy(ctxsb[:, m, :], cp[:, :R])

            # S_un^T [91, R] from unnormalized aw -> Ssu bf16 (time-major)
            sp = psB.tile([NOBJ, R], dt.float32, tag="big")
            for b in range(B):
                nc.tensor.matmul(
                    sp[:, b * T : (b + 1) * T],
                    oh_sb[:, b * NOBJ : (b + 1) * NOBJ],
                    Esb[:, b * T : (b + 1) * T],
                )
            nc.vector.tensor_copy(
                Ssu[:, 0:R].rearrange("p (t bb) -> p t bb", bb=B),
                sp[:, :].rearrange("p (bb t) -> p t bb", t=T),
            )

            # p_pre = (pge@ctx_un)*rinv (batch-major) + pgd@H (time-major).
            en = pch[:, 0:R]        # later reused for sscl
            den = pch[:, R : 2 * R]
            p_ = pch[:, 2 * R : 3 * R]
            q1 = pch[:, 3 * R : 4 * R]
            phs = pch[:, 4 * R : 5 * R]
            pp = psB.tile([1, R], dt.float32, tag="big")
            ph = psB.tile([1, R], dt.float32, tag="big")
            for n0 in range(0, R, 512):
                nn = min(512, R - n0)
                for k in range(2):
                    nc.tensor.matmul(
                        pp[:, n0 : n0 + nn],
                        pge_sb[:, k : k + 1],
                        ctxsb[:, k, n0 : n0 + nn],
                        start=(k == 0),
                        stop=(k == 1),
                    )
                for k in range(4):
                    nc.tensor.matmul(
                        ph[:, n0 : n0 + nn],
                        pgd_sb[:, k : k + 1],
                        HTv[:, k, :][:, n0 : n0 + nn],
                        start=(k == 0),
                        stop=(k == 3),
                    )
            nc.vector.tensor_mul(en, pp[:, :], rinv[:])
            # phs (batch-major) = permute(ph) :  phs[b*T+t] = ph[t*B+b]
            nc.vector.tensor_copy(
                phs.rearrange("o (bb t) -> o bb t", t=T),
                ph[:, :].rearrange("o (t bb) -> o bb t", bb=B),
            )
            nc.vector.tensor_add(den, en, phs)
            # p = sigmoid(ppre + pb); table switch to the sigmoid set is fine
            # here (no exp needed afterwards)
            nc.scalar.activation(p_, den, AF.Sigmoid, bias=pb_sb[0:1, 0:1])
            # q1 = 1 - p
            nc.vector.tensor_scalar(
                out=q1, in0=p_, scalar1=-1.0, scalar2=1.0,
                op0=ALU.mult, op1=ALU.add,
            )

            # permute p (batch-major) -> time-major bf16 row ptm
            nc.vector.tensor_copy(
                ptm[:, 0:R].rearrange("o (t bb) -> o t bb", bb=B),
                p_.rearrange("o (bb t) -> o t bb", t=T),
            )
            if RP > R:
                nc.vector.memset(ptm[:, R:RP], 0.0)
            # sscl = rinv * (1-p)  (batch-major) -> sstm f32 (time-major)
            nc.vector.tensor_mul(en, rinv[:], q1)
            nc.vector.tensor_copy(
                sstm[:, 0:R].rearrange("o (t bb) -> o t bb", bb=B),
                en.rearrange("o (bb t) -> o t bb", t=T),
            )
            if RP > R:
                nc.vector.memset(sstm[:, R:RP], 0.0)

            # Hs = H^T * bcast(p_tm)  [128, 4, RP] bf16; bcast row 91 also
            # provides ptm on partition 91 for the Ss2 bias row.
            pbp = psB.tile([128, 1024], dt.float32, tag="big")
            for n0 in range(0, R, 512):
                nn = min(512, R - n0)
                nc.tensor.matmul(
                    pbp[:, n0 : n0 + nn], ones_bf[:], ptm[:, n0 : n0 + nn],
                )
            for k in range(4):
                nc.vector.tensor_mul(Hs[:, k, 0:R], HTv[:, k, :], pbp[:, :R])
                if RP > R:
                    nc.vector.memset(Hs[:, k, R:RP], 0.0)
            # p row lives at partition 96 (engine ops need 32-aligned
            # base partitions); rows 91:96 are zeroed on both operands
            nc.vector.memset(Ss2[64:96, 0:RP], 0.0)
            nc.scalar.copy(Ss2[96:97, 0:R], pbp[96:97, :R])
            if RP > R:
                nc.vector.memset(Ss2[96:97, R:RP], 0.0)

            # Ss2[0:91] = Ssu * bcast(sstm)   (both time-major)
            qbtm = psB.tile([NOBJ, R], dt.float32, tag="big")
            for n0 in range(0, R, 512):
                nn = min(512, R - n0)
                nc.tensor.matmul(
                    qbtm[:, n0 : n0 + nn],
                    ones_f[:, 0:NOBJ],
                    sstm[:, n0 : n0 + nn],
                )
            nc.vector.tensor_mul(Ss2[0:NOBJ, 0:R], Ssu[:, 0:R], qbtm[:, :R])
            if RP > R:
                nc.vector.memset(Ss2[0:NOBJ, R:RP], 0.0)

            # ---------- vocab matmul, vocab-sharded ----------
            for m in range(MT):
                rlo = m * 128
                rhi = min(R, rlo + 128)
                rn = rhi - rlo
                if rn <= 0:
                    break
                for n0 in range(0, VL, 512):
                    nn = min(512, VL - n0)
                    ps = psA.tile([128, 512], dt.float32, tag="psA")
                    for k in range(4):
                        nc.tensor.matmul(
                            ps[:rn, :nn],
                            Hs[:, k, rlo : rlo + rn],
                            Wt_sb[:, k, n0 : n0 + nn],
                            start=(k == 0),
                            stop=False,
                        )
                    # pointer logits + p*linear_b in one K=97 matmul
                    nc.tensor.matmul(
                        ps[:rn, :nn],
                        Ss2[:, rlo : rlo + rn],
                        conv_sb[:, n0 : n0 + nn],
                        start=False,
                        stop=True,
                    )
                    om = omp.tile([128, 512], dt.float32, tag="om")
                    nc.scalar.copy(om[:rn, :nn], ps[:rn, :nn])
                    nc.sync.dma_start(d_out[rlo:rhi, n0 : n0 + nn], om[:rn, :nn])

    n_fix = _split_multi_waits(nc, mybir, max_waits=1)
    nc.finalize()
    return nc


def _prep_inputs(features, captions, lengths, encoder_input, encoder_output,
                 embed_W, W_ih, W_hh, b_ih, b_hh, linear_W, linear_b,
                 attn_W, attn_b, pge_W, pge_b, pgd_W, pgd_b, converter):
    """Host-side sharding/layout prep. Returns per-core in_maps and T."""
    import ml_dtypes

    bf16 = ml_dtypes.bfloat16
    f32 = np.float32

    features = np.asarray(features, f32)
    captions = np.asarray(captions)
    encoder_input = np.asarray(encoder_input)
    encoder_output = np.asarray(encoder_output, f32)
    embed_W = np.asarray(embed_W, f32)
    W_ih = np.asarray(W_ih, f32)
    W_hh = np.asarray(W_hh, f32)
    b_ih = np.asarray(b_ih, f32)
    b_hh = np.asarray(b_hh, f32)
    linear_W = np.asarray(linear_W, f32)
    linear_b = np.asarray(linear_b, f32)
    attn_W = np.asarray(attn_W, f32)
    attn_b = np.asarray(attn_b, f32)
    pge_W = np.asarray(pge_W, f32)
    pge_b = np.asarray(pge_b, f32)
    pgd_W = np.asarray(pgd_W, f32)
    pgd_b = np.asarray(pgd_b, f32)
    converter = np.asarray(converter, f32)

    T = int(lengths)
    R = T * B

    # x sequence: t=0 -> features, t>=1 -> embed_W[captions[:, t-1]]
    emb = np.empty((B, T, EMB), f32)
    emb[:, 0, :] = features
    if T > 1:
        emb[:, 1:, :] = embed_W[captions[:, : T - 1]]
    # XT [EMB, R], column c = t*B + b
    XT = np.ascontiguousarray(emb.transpose(2, 1, 0).reshape(EMB, R))
    xt = XT.reshape(2, 128, R).astype(bf16)

    # gate permutation [i, f, o, g] (torch order is i, f, g, o)
    perm = np.r_[0:HID, HID:2 * HID, 3 * HID:4 * HID, 2 * HID:3 * HID]
    wih = np.ascontiguousarray(W_ih[perm].T).reshape(2, 128, 4 * HID).astype(bf16)
    whh = np.ascontiguousarray(W_hh[perm].T).reshape(4, 128, 4 * HID).astype(bf16)
    biasv = (b_ih + b_hh)[perm].astype(f32)
    biaspm = np.ascontiguousarray(biasv.reshape(16, 128).T)

    encb = np.ascontiguousarray(
        encoder_output.transpose(1, 0, 2).reshape(L, B * EMB)
    ).astype(bf16)
    enct = np.ascontiguousarray(
        encoder_output.transpose(2, 0, 1).reshape(2, 128, B * L)
    ).astype(f32)
    oh = np.ascontiguousarray(
        np.eye(NOBJ, dtype=f32)[encoder_input].transpose(1, 0, 2).reshape(L, B * NOBJ)
    )
    mask01 = np.ascontiguousarray((encoder_input.T != 0).astype(f32))  # [L, B]
    eye = np.eye(128, dtype=f32).astype(bf16)

    awt = np.ascontiguousarray(attn_W.T).reshape(4, 128, EMB).astype(bf16)
    attnb = np.ascontiguousarray(attn_b.reshape(2, 128).T).astype(f32)
    pge = np.ascontiguousarray(pge_W.reshape(EMB).reshape(2, 128).T).astype(bf16)
    pgd = np.ascontiguousarray(pgd_W.reshape(HID).reshape(4, 128).T).astype(bf16)
    pb = np.array([[float(pge_b.reshape(-1)[0] + pgd_b.reshape(-1)[0])]], f32)

    common = dict(
        xt=xt, wih=wih, biaspm=biaspm, whh=whh, encb=encb, enct=enct,
        oh=oh, mask01=mask01, eye=eye, awt=awt, attnb=attnb, pge=pge,
        pgd=pgd, pb=pb,
    )

    in_maps = []
    for c in range(NCORES):
        v0, v1 = c * VL, (c + 1) * VL
        wt = np.ascontiguousarray(linear_W[v0:v1].T).reshape(4, 128, VL).astype(bf16)
        # converter slice with linear_b appended as row 91 (paired with the
        # p_gen row of Ss2 on device)
        conv = np.ascontiguousarray(
            np.concatenate(
                [converter[:, v0:v1], np.zeros((5, VL), f32),
                 linear_b[v0:v1][None, :]], axis=0)
        ).astype(bf16)
        m = dict(common)
        m.update(wt=wt, conv=conv)
        in_maps.append(m)
    return in_maps, T, R


def kernel(**inputs):
    global LAST_RESULT
    from concourse.bass_utils import run_bass_kernel_spmd

    in_maps, T, R = _prep_inputs(**inputs)
    if T not in _CACHE:
        _CACHE[T] = _build(T)
    nc = _CACHE[T]

    res = run_bass_kernel_spmd(nc, in_maps, core_ids=list(range(NCORES)))
    LAST_RESULT = res
    out = np.concatenate([res.results[c]["out"] for c in range(NCORES)], axis=1)
    return out.astype(np.float32)
